# revision 72
# baseline (speedup 1.0000x reference)
"""BertEmbeddings (word lookup + header mean-pool scatter + pos/type/match
embeddings + TF-style LayerNorm) as a Bass/Tile kernel on 8 trn2 NeuronCores.

Sharding: data-parallel over batch (4 rows/core); embedding tables replicated.

Per-core device pipeline (v26):
  - index tiles land first on the SP HWDGE ring; big consts are [128, X]
    shaped and split across the SP/ACT rings so no single ring serializes
    them (a <128-partition tensor DMAs as a serial per-partition
    descriptor train; cB is zero-padded to 128 partitions for this reason)
  - bf16 word table. Word chunks 0,1 are gathered with BUILT-IN per-column
    indirect DMAs: the SWDGE gather-ucode library (DMAGatherAnt) takes a
    ~9-14us blocking load on GpSimd during which no GpSimd instruction -
    not even a built-in - can execute, so the first half of the word data
    flows while that load (auto-inserted before the first DMAGatherAnt)
    is in flight. Word chunks 2,3 and the compacted pair-packed header
    chunks then use DMAGatherAnt (~4ns/row descgen vs ~10.7ns/row for
    indirect) on SWDGE queues 1-3
  - per 128-token block: small-table adds via bf16 multihot matmuls into
    PSUM (+ pos rows via eye-matmul for the last chunk; DVE adds them for
    earlier chunks); emb(bf16) = words + psum via scalar_tensor_tensor
    whose accum_out yields the LN row-sum for free; sum-of-squares on ACT;
    stats batched per batch row; LN applies: ACT for chunk 0 (early, while
    GpSimd is walled), DVE elsewhere (~390ns each), GpSimd for two of the
    last chunk's blocks; outputs stored in bf16 (host widens to f32), one
    batched 4-block store per batch row alternating the two HWDGE rings -
    except the last row, stored per block so each slice flies as soon as
    its apply finishes
  - target block: header rows pair-added on DVE, segment-mean reduced by
    PE matmuls (one-hot matrix carries 1/len) accumulated with the
    targets' small-table multihot AND their pos rows (one-hot matmul
    against the resident pos matrix) in PSUM - deliberately NOT ordered
    after the main matmuls, so the const-only ones fill PE's idle window;
    bn_stats LayerNorm straight out of PSUM; result stored compact to
    out_t - the host places the rows at col_pos during unshard (skipping
    len==0 slots), so no device-side scatter waits on the block stores

All data-dependent arithmetic on embedding VALUES runs on device; the host
only reformats index tensors, precomputes input-independent table
derivatives (zero-row padding, one-hot/multi-hot encodings, 1/len weights),
and moves/widens device-computed rows while unsharding.
"""

import numpy as np

B, S, H = 32, 512, 768
VOCAB = 30522
NCORES = 8
BPC = B // NCORES            # batch rows per core
T = BPC * S                  # tokens per core
C, L = 32, 16                # columns, max header len
NSLOT = BPC * C              # 128 slots per core
ZROW = VOCAB                 # zero row in augmented word table
WROWS = VOCAB + 1
NV = 19                      # 2 + 11 + 6 small-table rows
EPS = 1e-12

# cB column layout: [mh(T) | small(768) | mht(128)]
CB_MH = 0
CB_SM = T
CB_MHT = T + H
CB_W = CB_MHT + 128
# cC column layout: [eye(128) | Mp(Gp*128) | poshot(4*128)]
CC_EYE = 0
CC_M = 128

_NC_CACHE = {}


def _build_nc(skip_affine: bool, Gp: int):
    from contextlib import ExitStack

    import concourse.bacc as bacc
    import concourse.tile as tile
    from concourse import mybir

    BF16 = mybir.dt.bfloat16
    I16 = mybir.dt.int16
    F32 = mybir.dt.float32

    nc = bacc.Bacc(
        "TRN2", target_bir_lowering=False, debug=False, num_swdge_queues=4,
        dynamic_dma_scratch_size=12288,
    )
    t = {}

    def inp(name, shape, dt=F32):
        t[name] = nc.dram_tensor(name, shape, dt, kind="ExternalInput").ap()

    inp("word_aug", [WROWS, H], BF16)
    inp("idx32w", [128, 8], mybir.dt.int32)  # word row idx chunks 0,1
    inp("idx16", [128, 128 + Gp * 16], I16)  # [widx16 | hidx pairs], wrap16
    inp("posm", [128, 4 * H], BF16)          # pos table, row s at [s%128, s//128]
    inp("cB", [128, CB_W], BF16)             # multihot | small | tgt multihot
    inp("cC", [128, 128 + Gp * 128 + 512], BF16)  # eye | seg matrix | poshot
    if not skip_affine:
        inp("lnw", [1, H])
        inp("lnb", [1, H])
    out = nc.dram_tensor("out", [BPC, S, H], BF16, kind="ExternalOutput").ap()
    out_t = nc.dram_tensor("out_t", [128, H], F32, kind="ExternalOutput").ap()

    with tile.TileContext(nc) as tc, ExitStack() as ctx:
        _body(ctx, tc, t, out, out_t, skip_affine, Gp, mybir)
    nc.compile()
    return nc


def _body(ctx, tc, t, out, out_t, skip_affine, Gp, mybir):
    import concourse.bass as bass
    from concourse.tile import add_dep_helper

    nc = tc.nc
    F32 = mybir.dt.float32
    BF16 = mybir.dt.bfloat16
    MUL = mybir.AluOpType.mult
    ADD = mybir.AluOpType.add
    SUB = mybir.AluOpType.subtract
    AF = mybir.ActivationFunctionType

    const = ctx.enter_context(tc.tile_pool(name="const", bufs=1))
    setup = ctx.enter_context(tc.tile_pool(name="setup", bufs=1))
    hpool = ctx.enter_context(tc.tile_pool(name="hdr", bufs=1))
    wpool = ctx.enter_context(tc.tile_pool(name="wrd", bufs=4))
    epool = ctx.enter_context(tc.tile_pool(name="emb", bufs=16))
    opool = ctx.enter_context(tc.tile_pool(name="outp", bufs=4))
    qpool = ctx.enter_context(tc.tile_pool(name="sq", bufs=2))
    spool = ctx.enter_context(tc.tile_pool(name="stat", bufs=4))
    psum = ctx.enter_context(tc.tile_pool(name="ps", bufs=3, space="PSUM"))
    psumt = ctx.enter_context(tc.tile_pool(name="pst", bufs=1, space="PSUM"))

    # ---- index tiles first; alone on the SP HWDGE ring so they land fast.
    s_idx32w = const.tile([128, 8], mybir.dt.int32)
    nc.sync.dma_start(s_idx32w[:], t["idx32w"])
    s_idx16 = const.tile([128, 128 + Gp * 16], mybir.dt.int16)
    nc.sync.dma_start(s_idx16[:], t["idx16"])

    # ---- big consts split across the two HWDGE rings (parallel DMA)
    s_cB = const.tile([128, CB_W], BF16)
    nc.scalar.dma_start(s_cB[:], t["cB"])
    s_posm = const.tile([128, 4 * H], BF16)
    nc.sync.dma_start(s_posm[:], t["posm"])
    s_cC = const.tile([128, 128 + Gp * 128 + 512], BF16)
    nc.scalar.dma_start(s_cC[:], t["cC"])

    # ------- word chunk 0: per-column built-in indirect DMA. The ucode
    # library load (auto-inserted before the first DMAGatherAnt below) then
    # kicks at ~14.8us - after the const-load burst, so it runs at full DMA
    # speed - and chunk-0 compute covers most of the wall.
    wchs = [
        wpool.tile([128, 4, H], BF16, tag="wch", name=f"wch{ch}")
        for ch in range(BPC)
    ]
    word_gathers = []
    last_ind = None
    for ch in range(2):
        for jj in range(4):
            last_ind = nc.gpsimd.indirect_dma_start(
                wchs[ch][:, jj, :], None, t["word_aug"],
                bass.IndirectOffsetOnAxis(
                    ap=s_idx32w[:, 4 * ch + jj : 4 * ch + jj + 1], axis=0
                ),
            )

    # ------- word chunks 2,3: SWDGE gathers after the lib wall -----------
    wq = [None, None, 1, 2]
    for ch in range(2, BPC):
        g = nc.gpsimd.dma_gather(
            wchs[ch][:], t["word_aug"], s_idx16[:, 32 * ch : 32 * (ch + 1)],
            512, 512, H, queue_num=wq[ch],
        )
        if ch == 2:
            add_dep_helper(g.ins, last_ind.ins, sync=True,
                           reason="indirect bootstrap before lib wall")
        word_gathers.append(g)

    # ----- compacted pair-packed header gathers (SWDGE queues 1,2) -------
    GH1 = (2 * Gp + 1) // 2          # chunks in first header gather
    GH2 = 2 * Gp - GH1
    hch1 = hpool.tile([128, GH1, H], BF16, tag="h1")
    hg = nc.gpsimd.dma_gather(
        hch1[:], t["word_aug"], s_idx16[:, 128 : 128 + GH1 * 8],
        GH1 * 128, GH1 * 128, H, queue_num=3,
    )
    hdr_gathers = [hg]
    if GH2 > 0:
        hch2 = hpool.tile([128, GH2, H], BF16, tag="h2")
        g = nc.gpsimd.dma_gather(
            hch2[:], t["word_aug"],
            s_idx16[:, 128 + GH1 * 8 : 128 + 2 * Gp * 8],
            GH2 * 128, GH2 * 128, H, queue_num=1,
        )
        add_dep_helper(g.ins, word_gathers[0].ins, sync=False,
                       reason="SWDGE q1 order")
        hdr_gathers.append(g)

    def hcol(c):
        if c < GH1:
            return hch1[:, c, :]
        return hch2[:, c - GH1, :]

    s_eps = const.tile([128, 1], F32)
    nc.vector.memset(s_eps[:], EPS)

    if not skip_affine:
        s_lnw = const.tile([128, H], F32)
        gg = nc.gpsimd.dma_start(s_lnw[:], t["lnw"].partition_broadcast(128))
        add_dep_helper(gg.ins, hdr_gathers[-1].ins, sync=False,
                       reason="gathers first on SWDGE queue")
        s_lnb = const.tile([128, H], F32)
        nc.gpsimd.dma_start(s_lnb[:], t["lnb"].partition_broadcast(128))

    s_eye = s_cC[:, CC_EYE:CC_EYE + 128]
    s_small = s_cB[0:NV, CB_SM:CB_SM + H]
    CC_PH = 128 + Gp * 128

    # ---------------- main token blocks (no header dependency) -----------
    inv_h = 1.0 / H
    last_mm = None
    last_act = None
    store_eng = [nc.sync, nc.scalar, nc.sync, nc.scalar]
    for ch in range(BPC):
        wch = wchs[ch]
        usum = spool.tile([128, 4], F32, tag="usum")
        rsq = spool.tile([128, 4], F32, tag="rsq")
        embs = []
        for jj in range(4):
            j = ch * 4 + jj
            ps = psum.tile([128, H], F32, tag="ps")
            lhs_mh = s_cB[0:NV, CB_MH + j * 128 : CB_MH + (j + 1) * 128]
            pos_on_pe = ch == 3
            # keep matmuls with the same stationary adjacent (one
            # LDWEIGHTS per stationary instead of per matmul)
            for lo, hi in ((0, 512), (512, H)):
                nc.tensor.matmul(
                    ps[:, lo:hi], lhs_mh, s_small[:, lo:hi],
                    start=True, stop=not pos_on_pe,
                )
            if pos_on_pe:
                for lo, hi in ((0, 512), (512, H)):
                    last_mm = nc.tensor.matmul(
                        ps[:, lo:hi], s_eye,
                        s_posm[:, jj * H + lo : jj * H + hi],
                        start=False, stop=True,
                    )

            # emb = words (+ pos) + ps; accum_out feeds the LN mean
            emb = epool.tile([128, H], BF16)
            if pos_on_pe:
                wsrc = wch[:, jj, :]
            else:
                wsrc = epool.tile([128, H], BF16, tag="wp")
                nc.vector.tensor_add(
                    wsrc[:], wch[:, jj, :],
                    s_posm[:, jj * H : (jj + 1) * H],
                )
                wsrc = wsrc[:]
            nc.vector.scalar_tensor_tensor(
                emb[:], wsrc, 1.0, ps[:, 0:H],
                op0=MUL, op1=ADD, accum_out=usum[:, jj : jj + 1],
            )
            embs.append(emb)

            sq = qpool.tile([128, H], BF16)
            last_act = nc.scalar.activation(
                sq[:], emb[:], AF.Square, accum_out=rsq[:, jj : jj + 1]
            )

        # batched LN stats for the 4 blocks of this batch row
        uneg = spool.tile([128, 4], F32, tag="uneg")
        nc.vector.tensor_scalar_mul(uneg[:], usum[:], -inv_h)
        sq2 = spool.tile([128, 4], F32, tag="sq2")
        nc.vector.tensor_mul(sq2[:], uneg[:], uneg[:])
        var = spool.tile([128, 4], F32, tag="var")
        nc.vector.scalar_tensor_tensor(
            var[:], rsq[:], inv_h, sq2[:], op0=MUL, op1=SUB
        )
        rstd = spool.tile([128, 4], F32, tag="rstd")
        last_act = nc.scalar.activation(
            rstd[:], var[:], AF.Sqrt, bias=s_eps[:], scale=1.0
        )
        nc.vector.reciprocal(rstd[:], rstd[:])
        nub = spool.tile([128, 4], F32, tag="nub")
        nc.vector.tensor_mul(nub[:], uneg[:], rstd[:])

        o_ch = opool.tile([128, 4, H], BF16)
        for jj in range(4):
            o = o_ch[:, jj, :]
            # keep ACT out of the tail: it owns the squares. DVE applies
            # are cheap (~390ns bf16); GpSimd takes two once the lib wall
            # and descgen are past.
            if ch in (0, 2):
                last_act = nc.scalar.activation(
                    o, embs[jj][:], AF.Identity,
                    bias=nub[:, jj : jj + 1], scale=rstd[:, jj : jj + 1],
                )
            elif ch == 1 or jj >= 2:
                nc.vector.tensor_scalar(
                    o, embs[jj][:], rstd[:, jj : jj + 1],
                    nub[:, jj : jj + 1], op0=MUL, op1=ADD,
                )
            else:
                nc.gpsimd.tensor_scalar(
                    o, embs[jj][:], rstd[:, jj : jj + 1],
                    nub[:, jj : jj + 1], op0=MUL, op1=ADD,
                )
            if not skip_affine:
                nc.vector.tensor_mul(o, o, s_lnw[:])
                nc.vector.tensor_add(o, o, s_lnb[:])
        if ch < 3:
            # one batched store for the 4 blocks of this batch row
            dst = out.rearrange("b (j p) h -> b p j h", p=128)[ch]
            store_eng[ch].dma_start(dst, o_ch[:])
        else:
            # last batch row: store per block so each 128-row slice flies
            # as soon as its apply finishes instead of waiting for all 4
            for jj in range(4):
                store_eng[ch].dma_start(
                    out[ch, jj * 128 : (jj + 1) * 128, :], o_ch[:, jj, :]
                )

    # -------- target block: pooled headers + pos + small -----------------
    # pair-add same-slot header tokens (host packed them adjacently)
    hps = []
    for gp in range(Gp):
        hp = setup.tile([128, H], BF16, tag=f"hp{gp}")
        nc.vector.tensor_add(hp[:], hcol(2 * gp), hcol(2 * gp + 1))
        hps.append(hp)

    ps_t = psumt.tile([128, H], F32)
    # stationary-outer matmul order: one LDWEIGHTS per stationary
    mm = None
    for lo, hi in ((0, 512), (512, H)):
        m0 = nc.tensor.matmul(
            ps_t[:, lo:hi], s_cB[0:NV, CB_MHT:CB_MHT + 128],
            s_small[:, lo:hi], start=True, stop=False,
        )
        if mm is None:
            mm = m0
    # targets' pos rows via one-hot matmul against the pos matrix
    for jb in range(4):
        for lo, hi in ((0, 512), (512, H)):
            nc.tensor.matmul(
                ps_t[:, lo:hi],
                s_cC[:, CC_PH + jb * 128 : CC_PH + (jb + 1) * 128],
                s_posm[:, jb * H + lo : jb * H + hi],
                start=False, stop=False,
            )
    # segment mean-pool of the pair-added header rows (1/len in cC)
    for gp in range(Gp):
        for lo, hi in ((0, 512), (512, H)):
            nc.tensor.matmul(
                ps_t[:, lo:hi],
                s_cC[:, CC_M + gp * 128 : CC_M + (gp + 1) * 128],
                hps[gp][:, lo:hi],
                start=False, stop=(gp == Gp - 1),
            )

    stats = setup.tile([128, 2, 6], F32)
    nc.vector.bn_stats(stats[:, 0, :], ps_t[:, 0:384])
    nc.vector.bn_stats(stats[:, 1, :], ps_t[:, 384:768])
    mv = setup.tile([128, 2], F32)
    nc.vector.bn_aggr(mv[:], stats[:])
    rstd_t = setup.tile([128, 1], F32)
    nc.scalar.activation(
        rstd_t[:], mv[:, 1:2], AF.Sqrt, bias=s_eps[:], scale=1.0
    )
    nc.vector.reciprocal(rstd_t[:], rstd_t[:])
    o_t = setup.tile([128, H], F32)
    nc.vector.tensor_scalar(
        o_t[:], ps_t[:, 0:H], mv[:, 0:1], rstd_t[:], op0=SUB, op1=MUL
    )
    if not skip_affine:
        nc.vector.tensor_mul(o_t[:], o_t[:], s_lnw[:])
        nc.vector.tensor_add(o_t[:], o_t[:], s_lnb[:])

    # compact store; the host places these rows at col_pos while unsharding
    nc.sync.dma_start(out_t, o_t[:])


def _wrap16(flat):
    w = flat.reshape(-1, 16).T.astype(np.int16)
    return np.tile(w, (8, 1))


def _multihot(tt, mt, ti, n, dtype):
    mh1 = np.zeros((NV, n), dtype=dtype)
    ar = np.arange(n)
    mh1[tt, ar] = 1
    mh1[2 + mt, ar] += 1
    mh1[13 + ti, ar] += 1
    return mh1


def _prep_core(core, iid, hdr, tt, mt, ti, cpos, cidx, hlen, bf16):
    b0 = core * BPC
    sl = slice(b0, b0 + BPC)
    iids = iid[sl]

    # word row idx for chunks 0,1 in per-partition int32 layout
    idx32w = np.ascontiguousarray(iids[:2].reshape(8, 128).T).astype(np.int32)
    widx16 = _wrap16(iids.reshape(-1))

    bb = np.arange(BPC)[:, None]
    sel_hdr = hdr[sl][bb, cidx[sl]]                      # [BPC, C, L]
    sel_len = hlen[sl][bb, cidx[sl]]                     # [BPC, C]

    # pair-pack valid header tokens: each pair holds 2 tokens of one slot
    pairs = []                                           # (slot, tok0, tok1)
    tok = sel_hdr.reshape(NSLOT, L)
    lens = sel_len.reshape(NSLOT)
    for s in range(NSLOT):
        ln = int(lens[s])
        for i in range(0, ln, 2):
            t0 = int(tok[s, i])
            t1 = int(tok[s, i + 1]) if i + 1 < ln else ZROW
            pairs.append((s, t0, t1))

    valid = lens > 0
    posidx = cpos[sl].reshape(NSLOT).astype(np.int32)
    tgtrow = (bb * S + cpos[sl]).reshape(-1).astype(np.int32)

    ttf, mtf, tif = tt[sl].reshape(-1), mt[sl].reshape(-1), ti[sl].reshape(-1)
    mh = _multihot(ttf, mtf, tif, T, bf16)

    tt_t = tt[sl][bb, cpos[sl]].reshape(-1)
    mt_t = mt[sl][bb, cpos[sl]].reshape(-1)
    ti_t = ti[sl][bb, cpos[sl]].reshape(-1)
    mht = _multihot(tt_t, mt_t, ti_t, NSLOT, bf16)

    return idx32w, widx16, pairs, lens, posidx, tgtrow, valid, mh, mht


def make_in_maps(inputs):
    import ml_dtypes

    bf16 = ml_dtypes.bfloat16
    inp = {k: np.asarray(v) for k, v in inputs.items()}
    word = np.ascontiguousarray(inp["word_emb"], dtype=np.float32)
    word_aug = np.concatenate(
        [word.astype(bf16), np.zeros((1, H), bf16)], axis=0
    )

    small16 = np.concatenate(
        [inp["tok_type_emb"], inp["match_emb"], inp["type_emb"]], axis=0
    ).astype(np.float32).astype(bf16)                    # [19, H]

    pos16 = np.ascontiguousarray(inp["pos_emb"], dtype=np.float32).astype(bf16)
    posm = np.ascontiguousarray(
        pos16.reshape(4, 128, H).transpose(1, 0, 2).reshape(128, 4 * H)
    )
    eye = np.eye(128, dtype=bf16)

    lnw = np.ascontiguousarray(inp["ln_w"], dtype=np.float32).reshape(1, H)
    lnb = np.ascontiguousarray(inp["ln_b"], dtype=np.float32).reshape(1, H)
    skip_affine = bool(np.all(lnw == 1.0) and np.all(lnb == 0.0))

    iid = inp["input_ids"].astype(np.int64)
    hdr = inp["header_ids"].astype(np.int64)
    tt = inp["token_type_ids"].astype(np.int64)
    mt = inp["match_type_ids"].astype(np.int64)
    ti = inp["type_idx"].astype(np.int64)
    cpos = inp["col_pos"].astype(np.int64)
    cidx = inp["col_idx"].astype(np.int64)
    hlen = inp["header_len"].astype(np.int64)

    pre = [
        _prep_core(core, iid, hdr, tt, mt, ti, cpos, cidx, hlen, bf16)
        for core in range(NCORES)
    ]
    # static pair-group count shared by all cores (compiled in)
    Gp = max(1, max((len(p[2]) + 127) // 128 for p in pre))

    in_maps = []
    scat = []
    for core, (idx32w, widx16, pairs, lens, posidx, tgtrow, valid, mh, mht) in enumerate(pre):
        hflat = np.full(2 * Gp * 128, ZROW, np.int64)
        M = np.zeros((128, Gp * 128), dtype=np.float32)
        for q, (s, t0, t1) in enumerate(pairs):
            p, gp = q % 128, q // 128
            hflat[2 * gp * 128 + p] = t0
            hflat[(2 * gp + 1) * 128 + p] = t1
            M[p, gp * 128 + s] = 1.0 / max(int(lens[s]), 1)
        hidx = _wrap16(hflat)
        idx16 = np.ascontiguousarray(
            np.concatenate([widx16, hidx], axis=1)
        )

        poshot = np.zeros((128, 4 * 128), dtype=np.float32)
        slot = np.arange(NSLOT)
        poshot[posidx % 128, (posidx // 128) * 128 + slot] = 1.0

        cB = np.zeros((128, CB_W), dtype=bf16)
        cB[:NV] = np.concatenate([mh, small16, mht], axis=1)
        cC = np.ascontiguousarray(np.concatenate(
            [eye, M.astype(bf16), poshot.astype(bf16)], axis=1
        ))
        m = dict(
            word_aug=word_aug, idx32w=idx32w, idx16=idx16, posm=posm,
            cB=cB, cC=cC,
        )
        if not skip_affine:
            m["lnw"] = lnw
            m["lnb"] = lnb
        in_maps.append(m)
        scat.append((tgtrow, valid))
    return in_maps, skip_affine, Gp, scat


def get_nc(skip_affine, Gp):
    key = (skip_affine, Gp)
    if key not in _NC_CACHE:
        _NC_CACHE[key] = _build_nc(skip_affine, Gp)
    return _NC_CACHE[key]


def run_hw(inputs, trace=False, trace_cores=None):
    """Returns (out [B,S,H] f32, BassKernelResults)."""
    from concourse.bass_utils import run_bass_kernel_spmd

    in_maps, skip_affine, Gp, scat = make_in_maps(inputs)
    nc = get_nc(skip_affine, Gp)
    res = run_bass_kernel_spmd(
        nc, in_maps, core_ids=list(range(NCORES)), trace=trace,
        trace_cores=trace_cores,
    )
    outs = []
    for c in range(NCORES):
        o = np.asarray(res.results[c]["out"]).astype(np.float32)
        o = o.reshape(T, H)
        tgtrow, valid = scat[c]
        o[tgtrow[valid]] = np.asarray(res.results[c]["out_t"])[valid]
        outs.append(o.reshape(BPC, S, H))
    out = np.concatenate(outs, axis=0)
    return out, res


def kernel(**inputs) -> np.ndarray:
    out, _ = run_hw(inputs, trace=False)
    return out
